# revision 14
# baseline (speedup 1.0000x reference)
"""Trainium2 Bass kernel for batched 64-point DCT (flattened-patch GEMM).

Reference computation: out = x.reshape(b, -1, 64) @ K, reshaped back.
Pure data parallel over 8 NeuronCores: core i handles batch i as a
[49152, 64] x [64, 64] GEMM.

The problem is HBM-bandwidth bound (per core: 12 MiB in + 12 MiB out in
fp32).  The 2e-2 relative-error budget admits bf16 for both the input
stream and the output stream (measured rel err 4.2e-3), halving HBM
traffic to ~12.6 MB/core against the ~360 GB/s per-core DMA roofline.

Device layout (host-prepared, all bf16):
  x[(z*64+s), pair] = inp[2*pair+z, s]   -- [128, 24576]
  kblk = blockdiag(K, K)                 -- [128, 128], stationary
  y[(z*64+f), pair] = out[2*pair+z, f]   -- [128, 24576]

Per 2048-pair tile: one input DMA (4 KB/partition), four matmuls
(kblk.T @ x_chunk -> PSUM [128, 512] fp32, one full bank each), four
PSUM->SBUF bf16 copies alternating DVE/ACT, one output DMA.  Input
rides the SP hwdge queue (which it saturates at ~230 GB/s); stores
alternate between the gpsimd swdge queue and the ACT hwdge queue so
the two directions overlap at the ~425 GB/s aggregate DMA ceiling.
"""

import numpy as np
import ml_dtypes

import concourse.mybir as mybir
from concourse import bacc
from concourse.bass_utils import run_bass_kernel_spmd
from concourse.tile import TileContext

P = 128       # SBUF partitions / blockdiag contraction dim
S = 64        # DCT size
N_CORES = 8
TILE_PAIRS = 2048          # pair-columns per macro-tile
CHUNK = 512                # pair-columns per matmul (one PSUM bank)
BF16 = mybir.dt.bfloat16


def build_kernel(n_patches: int):
    n_pairs = n_patches // 2
    assert n_pairs % TILE_PAIRS == 0
    n_tiles = n_pairs // TILE_PAIRS
    nc = bacc.Bacc(
        "TRN2",
        target_bir_lowering=False,
        debug=False,
        enable_asserts=False,
        num_devices=N_CORES,
    )
    x = nc.dram_tensor("x", [P, n_pairs], BF16, kind="ExternalInput")
    k = nc.dram_tensor("k", [P, P], BF16, kind="ExternalInput")
    y = nc.dram_tensor("y", [P, n_pairs], BF16, kind="ExternalOutput")

    xv = x.ap().rearrange("r (t n) -> t r n", n=TILE_PAIRS)
    yv = y.ap().rearrange("r (t n) -> t r n", n=TILE_PAIRS)

    with TileContext(nc) as tc:
        with (
            tc.tile_pool(name="consts", bufs=1) as consts,
            tc.tile_pool(name="xin", bufs=6) as x_pool,
            tc.tile_pool(name="outsb", bufs=6) as out_pool,
            tc.tile_pool(name="pout", bufs=8, space="PSUM") as pout_pool,
        ):
            kblk = consts.tile([P, P], BF16)
            first_x = x_pool.tile(
                [P, TILE_PAIRS], BF16, tag="x_tile", name="x_head"
            )
            # tile-0 load heads the Sync queue; kblk rides the Scalar queue.
            nc.sync.dma_start(out=first_x[:], in_=xv[0])
            nc.scalar.dma_start(out=kblk[:], in_=k.ap())

            for ti in range(n_tiles):
                if ti == 0:
                    x_tile = first_x
                else:
                    x_tile = x_pool.tile(
                        [P, TILE_PAIRS], BF16, tag="x_tile",
                        name=f"x_body{ti}",
                    )
                    nc.sync.dma_start(out=x_tile[:], in_=xv[ti])
                out_sb = out_pool.tile([P, TILE_PAIRS], BF16)
                for j in range(TILE_PAIRS // CHUNK):
                    po = pout_pool.tile([P, CHUNK], mybir.dt.float32)
                    nc.tensor.matmul(
                        po[:],
                        lhsT=kblk[:],
                        rhs=x_tile[:, CHUNK * j : CHUNK * (j + 1)],
                        start=True,
                        stop=True,
                    )
                    # gpsimd cannot read PSUM; split copies over DVE/ACT
                    if j % 2 == 0:
                        nc.vector.tensor_copy(
                            out_sb[:, CHUNK * j : CHUNK * (j + 1)], po[:]
                        )
                    else:
                        nc.scalar.copy(
                            out_sb[:, CHUNK * j : CHUNK * (j + 1)], po[:]
                        )
                # only SP/Activation/Pool queues may initiate DMAs; stores
                # alternate between the otherwise-idle gpsimd queue and scalar
                (nc.gpsimd if ti % 2 == 0 else nc.scalar).dma_start(
                    out=yv[ti], in_=out_sb[:]
                )
    nc.compile()
    return nc


def prep_inputs(x_full: np.ndarray, kmat: np.ndarray):
    """Full [8, C, H, W] fp32 -> per-core device in_maps (bf16)."""
    b = x_full.shape[0]
    n_patches = x_full[0].size // S
    n_pairs = n_patches // 2
    x16 = x_full.astype(ml_dtypes.bfloat16)
    # [b, n_pairs, 2, 64] -> [b, (z s), pair]
    xt = np.ascontiguousarray(
        x16.reshape(b, n_pairs, 2 * S).transpose(0, 2, 1)
    ).reshape(b, P, n_pairs)
    kblk = np.zeros((P, P), dtype=ml_dtypes.bfloat16)
    kblk[:S, :S] = kmat.astype(ml_dtypes.bfloat16)
    kblk[S:, S:] = kmat.astype(ml_dtypes.bfloat16)
    return [{"x": xt[i], "k": kblk} for i in range(b)]


def unshard_output(res_results, shape):
    """Per-core y [128, n_pairs] bf16 -> full fp32 [8, C, H, W]."""
    b, c, h, w = shape
    n_pairs = c * h * w // S // 2
    outs = []
    for i in range(b):
        yv = np.asarray(res_results[i]["y"]).reshape(2, S, n_pairs)
        # out[2p+z, f] = y[(z f), p]
        o = yv.transpose(2, 0, 1).astype(np.float32).reshape(c, h, w)
        outs.append(o)
    return np.stack(outs, axis=0)


def kernel(inputs, kernel):
    x_full = np.asarray(inputs, dtype=np.float32)
    kmat = np.asarray(kernel, dtype=np.float32)
    b, c, h, w = x_full.shape
    assert b == N_CORES, f"expected batch {N_CORES}, got {b}"
    n_patches = c * h * w // S
    nc = build_kernel(n_patches)
    in_maps = prep_inputs(x_full, kmat)
    res = run_bass_kernel_spmd(nc, in_maps, core_ids=list(range(N_CORES)))
    return unshard_output(res.results, (b, c, h, w))


# revision 16
# speedup vs baseline: 1.0006x; 1.0006x over previous
"""Trainium2 Bass kernel for batched 64-point DCT (flattened-patch GEMM).

Reference computation: out = x.reshape(b, -1, 64) @ K, reshaped back.
Pure data parallel over 8 NeuronCores: core i handles batch i as a
[49152, 64] x [64, 64] GEMM.

The problem is HBM-bandwidth bound (per core: 12 MiB in + 12 MiB out in
fp32).  The 2e-2 relative-error budget admits bf16 for both the input
stream and the output stream (measured rel err 4.2e-3), halving HBM
traffic to ~12.6 MB/core against the ~360 GB/s per-core DMA roofline.

Device layout (host-prepared, all bf16):
  x[(z*64+s), pair] = inp[2*pair+z, s]   -- [128, 24576]
  kblk = blockdiag(K, K)                 -- [128, 128], stationary
  y[(z*64+f), pair] = out[2*pair+z, f]   -- [128, 24576]

Per 2048-pair tile: one input DMA (4 KB/partition), four matmuls
(kblk.T @ x_chunk -> PSUM [128, 512] fp32, one full bank each), four
PSUM->SBUF bf16 copies alternating DVE/ACT, one output DMA.  Input
rides the SP hwdge queue (which it saturates at ~230 GB/s); stores
alternate between the gpsimd swdge queue and the ACT hwdge queue so
the two directions overlap at the ~425 GB/s aggregate DMA ceiling.
"""

import numpy as np
import ml_dtypes

import concourse.mybir as mybir
from concourse import bacc
from concourse.bass_utils import run_bass_kernel_spmd
from concourse.tile import TileContext

P = 128       # SBUF partitions / blockdiag contraction dim
S = 64        # DCT size
N_CORES = 8
TILE_PAIRS = 2048          # pair-columns per macro-tile
CHUNK = 512                # pair-columns per matmul (one PSUM bank)
BF16 = mybir.dt.bfloat16


def build_kernel(n_patches: int):
    n_pairs = n_patches // 2
    assert n_pairs % TILE_PAIRS == 0
    n_tiles = n_pairs // TILE_PAIRS
    nc = bacc.Bacc(
        "TRN2",
        target_bir_lowering=False,
        debug=False,
        enable_asserts=False,
        num_devices=N_CORES,
    )
    x = nc.dram_tensor("x", [P, n_pairs], BF16, kind="ExternalInput")
    k = nc.dram_tensor("k", [P, P], BF16, kind="ExternalInput")
    y = nc.dram_tensor("y", [P, n_pairs], BF16, kind="ExternalOutput")

    xv = x.ap().rearrange("r (t n) -> t r n", n=TILE_PAIRS)
    yv = y.ap().rearrange("r (t n) -> t r n", n=TILE_PAIRS)
    yap = y.ap()

    with TileContext(nc) as tc:
        with (
            tc.tile_pool(name="consts", bufs=1) as consts,
            tc.tile_pool(name="xin", bufs=6) as x_pool,
            tc.tile_pool(name="outsb", bufs=6) as out_pool,
            tc.tile_pool(name="pout", bufs=8, space="PSUM") as pout_pool,
        ):
            kblk = consts.tile([P, P], BF16)
            first_x = x_pool.tile(
                [P, TILE_PAIRS], BF16, tag="x_tile", name="x_head"
            )
            # tile-0 load heads the Sync queue; kblk rides the Scalar queue.
            nc.sync.dma_start(out=first_x[:], in_=xv[0])
            nc.scalar.dma_start(out=kblk[:], in_=k.ap())

            for ti in range(n_tiles):
                if ti == 0:
                    x_tile = first_x
                else:
                    x_tile = x_pool.tile(
                        [P, TILE_PAIRS], BF16, tag="x_tile",
                        name=f"x_body{ti}",
                    )
                    nc.sync.dma_start(out=x_tile[:], in_=xv[ti])
                out_sb = out_pool.tile([P, TILE_PAIRS], BF16)
                # first/last tiles: store in two 1024-pair halves, each
                # issued as soon as its two copies land and on different
                # queues, so the output stream starts earlier and the final
                # drain runs on both queues in parallel
                split = ti == 0 or ti >= n_tiles - 2
                base = ti * TILE_PAIRS
                half = TILE_PAIRS // 2
                for j in range(TILE_PAIRS // CHUNK):
                    po = pout_pool.tile([P, CHUNK], mybir.dt.float32)
                    nc.tensor.matmul(
                        po[:],
                        lhsT=kblk[:],
                        rhs=x_tile[:, CHUNK * j : CHUNK * (j + 1)],
                        start=True,
                        stop=True,
                    )
                    # gpsimd cannot read PSUM; split copies over DVE/ACT
                    if j % 2 == 0:
                        nc.vector.tensor_copy(
                            out_sb[:, CHUNK * j : CHUNK * (j + 1)], po[:]
                        )
                    else:
                        nc.scalar.copy(
                            out_sb[:, CHUNK * j : CHUNK * (j + 1)], po[:]
                        )
                    if split and j == 1:
                        nc.gpsimd.dma_start(
                            out=yap[:, base : base + half],
                            in_=out_sb[:, :half],
                        )
                if split:
                    nc.scalar.dma_start(
                        out=yap[:, base + half : base + TILE_PAIRS],
                        in_=out_sb[:, half:],
                    )
                else:
                    # stores alternate between the otherwise-idle gpsimd
                    # queue and scalar
                    (nc.gpsimd if ti % 2 == 0 else nc.scalar).dma_start(
                        out=yv[ti], in_=out_sb[:]
                    )
    nc.compile()
    return nc


def prep_inputs(x_full: np.ndarray, kmat: np.ndarray):
    """Full [8, C, H, W] fp32 -> per-core device in_maps (bf16)."""
    b = x_full.shape[0]
    n_patches = x_full[0].size // S
    n_pairs = n_patches // 2
    x16 = x_full.astype(ml_dtypes.bfloat16)
    # [b, n_pairs, 2, 64] -> [b, (z s), pair]
    xt = np.ascontiguousarray(
        x16.reshape(b, n_pairs, 2 * S).transpose(0, 2, 1)
    ).reshape(b, P, n_pairs)
    kblk = np.zeros((P, P), dtype=ml_dtypes.bfloat16)
    kblk[:S, :S] = kmat.astype(ml_dtypes.bfloat16)
    kblk[S:, S:] = kmat.astype(ml_dtypes.bfloat16)
    return [{"x": xt[i], "k": kblk} for i in range(b)]


def unshard_output(res_results, shape):
    """Per-core y [128, n_pairs] bf16 -> full fp32 [8, C, H, W]."""
    b, c, h, w = shape
    n_pairs = c * h * w // S // 2
    outs = []
    for i in range(b):
        yv = np.asarray(res_results[i]["y"]).reshape(2, S, n_pairs)
        # out[2p+z, f] = y[(z f), p]
        o = yv.transpose(2, 0, 1).astype(np.float32).reshape(c, h, w)
        outs.append(o)
    return np.stack(outs, axis=0)


def kernel(inputs, kernel):
    x_full = np.asarray(inputs, dtype=np.float32)
    kmat = np.asarray(kernel, dtype=np.float32)
    b, c, h, w = x_full.shape
    assert b == N_CORES, f"expected batch {N_CORES}, got {b}"
    n_patches = c * h * w // S
    nc = build_kernel(n_patches)
    in_maps = prep_inputs(x_full, kmat)
    res = run_bass_kernel_spmd(nc, in_maps, core_ids=list(range(N_CORES)))
    return unshard_output(res.results, (b, c, h, w))
